# revision 22
# baseline (speedup 1.0000x reference)
"""Multi-head attention (qkv proj + softmax attention + out proj) on 8 trn2 cores.

Sharding: zero-collective. Core c handles batch b=c//2 and query-half h=c%2
(1024 queries). The host passes x[b] ROTATED by h*1024 tokens so that each
core's queries are always local tokens 0..1023 while K/V still cover all 2048
tokens (key order is irrelevant to softmax). Host concatenates the 8 output
slabs [1024, 1024] into [4, 2048, 1024].

Per-core kernel (all on one NeuronCore, Tile-scheduled):
  1. PE-transpose x and the weights to contraction-major layout (fp32 in,
     bf16 out via the DVE PSUM-evacuation copy). x casts ride the (otherwise
     idle) ScalarE; V-projection weights load between the two x halves.
  2. QKV projection in bf16: QT/KT stored feature-major [d, t] so each
     128-row tile holds a PAIR of heads (2 x 64 d-rows); V stored token-major
     with interleaved ones columns (fused softmax-sum rows come out of the
     AV matmul for free).
  3. Attention per head-pair: QK^T into PSUM, softmax WITHOUT
     max-subtraction (scores are ~N(0,1); fp32 exp is safe), exp on ScalarE
     straight out of PSUM, AV+sums fused, fast approximate reciprocal on DVE.
  4. Out-projection in bf16 + bias, fp32 result.

Scheduling: engines execute in per-engine EMISSION order, so all overlap is
explicit software pipelining. The exp stream on ScalarE (2218ns per key
chunk) outpaces this pair's scores+AV matmuls (1720ns/chunk), so the NEXT
pair's weight transposes and K/Q projection are chopped into ~4-matmul units
and paced one-per-chunk through the attention emission; AV trails the
scores/exp stream by two chunks so probs are ready when AV issues. Pair 7
uses the out-projection weight/bias prep as its filler. This keeps the PE
gap-free through the whole mid-kernel (measured ~90% TensorE busy).
"""

import numpy as np

B, N, C = 4, 2048, 1024
H, D = 16, 64
P = 128
CG = C // P            # 8 contraction groups
TG = N // P            # 16 key-token chunks
TQ = N // 2            # 1024 queries per core
QB = 512               # query block (psum bank)
NB = TQ // QB          # 2
NPAIR = H // 2         # 8 head pairs
SCALE = 1.0 / np.sqrt(D).astype(np.float32)
DEBUG_DUMPS = False

_CACHE = {}


def _build():
    import concourse.tile as tile
    from concourse import bacc, mybir

    f32 = mybir.dt.float32
    nc = bacc.Bacc(
        "TRN2", target_bir_lowering=False, debug=False, num_devices=8
    )
    x_h = nc.dram_tensor("x", [N, C], f32, kind="ExternalInput").ap()
    wqkv_h = nc.dram_tensor("w_qkv", [3 * C, C], f32, kind="ExternalInput").ap()
    wout_h = nc.dram_tensor("w_out", [C, C], f32, kind="ExternalInput").ap()
    bout_h = nc.dram_tensor("b_out", [C], f32, kind="ExternalInput").ap()
    y_h = nc.dram_tensor("y", [TQ, C], f32, kind="ExternalOutput").ap()

    with tile.TileContext(nc) as tc:
        _emit(tc, x_h, wqkv_h, wout_h, bout_h, y_h)
    nc.compile()
    return nc


def _emit(tc, x_h, wqkv_h, wout_h, bout_h, y_h):
    from contextlib import ExitStack

    from concourse import mybir
    from concourse.masks import make_identity

    f32 = mybir.dt.float32
    bf16 = mybir.dt.bfloat16
    AF = mybir.ActivationFunctionType
    nc = tc.nc

    with ExitStack() as ctx:
        # ---------------- pools ----------------
        const = ctx.enter_context(tc.tile_pool(name="const", bufs=1))
        big = ctx.enter_context(tc.tile_pool(name="big", bufs=1))
        land = ctx.enter_context(tc.tile_pool(name="land", bufs=3))
        cstp = ctx.enter_context(tc.tile_pool(name="cstp", bufs=7))
        ktp = ctx.enter_context(tc.tile_pool(name="ktp", bufs=2))
        qtp = ctx.enter_context(tc.tile_pool(name="qtp", bufs=2))
        wtq = ctx.enter_context(tc.tile_pool(name="wtq", bufs=4))
        wtv = ctx.enter_context(tc.tile_pool(name="wtv", bufs=2))
        ptp = ctx.enter_context(tc.tile_pool(name="ptp", bufs=13))
        recp = ctx.enter_context(tc.tile_pool(name="recp", bufs=2))
        outp = ctx.enter_context(tc.tile_pool(name="outp", bufs=2))
        ps_pool = ctx.enter_context(
            tc.tile_pool(name="ps_pool", bufs=2, space="PSUM")
        )
        st_ps = ctx.enter_context(
            tc.tile_pool(name="st_ps", bufs=2, space="PSUM")
        )
        at_ps = ctx.enter_context(
            tc.tile_pool(name="at_ps", bufs=2, space="PSUM")
        )

        # ---------------- constants ----------------
        ident = const.tile([P, P], bf16)
        make_identity(nc, ident)
        onesrow = const.tile([1, P], f32)
        nc.gpsimd.memset(onesrow, 1.0)

        # ---------------- persistent tensors ----------------
        # V layout per pair p: cols [192p,192p+64)=V_h2p, [192p+64,+128)=ones,
        # [192p+128,+192)=V_h2p+1. The AV stationary operand for head A is
        # cols [192p,192p+128) = [V_A | ones] (output rows 0:64 = attention
        # out, rows 64:128 = softmax sums, replicated); for head B it is cols
        # [192p+64,192p+192) = [ones | V_B] (sums on rows 0:64, attention out
        # on rows 64:128). The ones block is shared between the two heads.
        xT = [big.tile([P, N], bf16, name=f"xT{g}") for g in range(CG)]
        V = [big.tile([P, 192 * NPAIR], bf16, name=f"V{i}") for i in range(TG)]
        aoT = [big.tile([P, TQ], bf16, name=f"aoT{g}") for g in range(CG)]
        for i in range(TG):
            v3i = V[i].rearrange("p (q e) -> p q e", e=64)
            nc.vector.memset(v3i[:, 1 : 3 * NPAIR : 3], 1.0)

        # -------- load x (fp32), cast bf16, PE-transpose into xT --------
        def load_cast(src_ap, name, scale=None, on_scalar=False):
            t = land.tile([P, C], f32, tag="land", name=f"ld{name}")
            nc.sync.dma_start(t, src_ap)
            b = cstp.tile([P, C], bf16, tag="cst", name=f"cs{name}")
            if on_scalar:
                if scale is None:
                    nc.scalar.copy(b, t)
                else:
                    nc.scalar.mul(b, t, scale)
            elif scale is None:
                nc.vector.tensor_copy(b, t)
            else:
                nc.vector.tensor_scalar_mul(b, t, scale)
            return b

        def w_chunk_loads(src_h, row0):
            return [
                load_cast(
                    src_h[(row0 + jj) * P : (row0 + jj + 1) * P, :],
                    f"w{row0}_{jj}",
                )
                for jj in range(4)
            ]

        def w_chunk_transpose(wn, pool, tag):
            wt = pool.tile([P, CG, 4 * P], bf16, tag=tag)
            for g in range(CG):
                psf = ps_pool.tile([P, QB], f32, tag="ps")
                ps = psf[:, :].bitcast(bf16)[:, 0 : 4 * P]
                for jj in range(4):
                    nc.tensor.transpose(
                        ps[:, jj * P : (jj + 1) * P],
                        wn[jj][:, g * P : (g + 1) * P],
                        ident,
                    )
                nc.vector.tensor_copy(wt[:, g, :], ps)
            return wt

        def w_chunk_T(src_h, row0, pool, tag):
            """Load+transpose 4 consecutive 128-row chunks of a [*, C] fp32
            weight into a [P, CG, 512] bf16 tile (contraction-major)."""
            return w_chunk_transpose(w_chunk_loads(src_h, row0), pool, tag)

        # x casts go on the scalar engine: it is idle until the first exp,
        # which parallelizes the cast chain with the psum-evac copies on DVE.
        # The V-projection weight DMAs are interleaved between the two x
        # halves so their prep overlaps the x-load tail instead of queueing
        # behind all 8.4MB of x.
        xb16 = [
            load_cast(x_h[i * P : (i + 1) * P, :], f"x{i}", on_scalar=True)
            for i in range(8)
        ]
        wv0_ld = w_chunk_loads(wqkv_h, 16)
        xb16 += [
            load_cast(x_h[i * P : (i + 1) * P, :], f"x{i}", on_scalar=True)
            for i in range(8, TG)
        ]

        # [128,128] bf16 PE transposes batched into one psum bank, one copy.
        # Smaller first batches so the PE starts as soon as chunk 0 lands.
        def xpose(i4, nb):
            for g in range(CG):
                psf = ps_pool.tile([P, QB], f32, tag="ps")
                ps = psf[:, :].bitcast(bf16)[:, 0 : nb * P]
                for k in range(nb):
                    nc.tensor.transpose(
                        ps[:, k * P : (k + 1) * P],
                        xb16[i4 + k][:, g * P : (g + 1) * P],
                        ident,
                    )
                nc.vector.tensor_copy(xT[g][:, i4 * P : (i4 + nb) * P], ps)

        # ---------------- V projection (interleaved with x prep) -----------
        def v_proj(fh, wvT, i0, i1):
            for i in range(i0, i1):
                ps = ps_pool.tile([P, QB], f32, tag="ps")
                for g in range(CG):
                    nc.tensor.matmul(
                        ps,
                        xT[g][:, i * P : (i + 1) * P],
                        wvT[:, g, :],
                        start=(g == 0),
                        stop=(g == CG - 1),
                    )
                # psum cols = v features [512*fh, 512*(fh+1)) = heads
                # 8fh..8fh+7. Scatter per-head 64-col blocks into the
                # interleaved V layout: head h -> col 192*(h//2)+128*(h%2).
                ps3 = ps.rearrange("p (k e) -> p k e", e=64)
                v3 = V[i].rearrange("p (q e) -> p q e", e=64)
                b0 = 12 * fh
                nc.vector.tensor_copy(v3[:, b0 : b0 + 12 : 3], ps3[:, 0::2])
                nc.vector.tensor_copy(
                    v3[:, b0 + 2 : b0 + 12 : 3], ps3[:, 1::2]
                )

        for i4, nb in ((0, 1), (1, 1), (2, 2), (4, 4)):
            xpose(i4, nb)
        wvT0 = w_chunk_transpose(wv0_ld, wtv, "wtv")
        wv1_ld = w_chunk_loads(wqkv_h, 20)
        v_proj(0, wvT0, 0, 8)
        xpose(8, 4)
        xpose(12, 4)
        wvT1 = w_chunk_transpose(wv1_ld, wtv, "wtv")
        v_proj(0, wvT0, 8, TG)
        v_proj(1, wvT1, 0, TG)

        # ---------------- per-pair: K proj, Q proj, attention ----------------
        # The exp on ScalarE is the rate limiter inside an attention block
        # (17.7us of exp vs 10.2us of scores+AV matmuls per (pair, tb)).
        # Since each engine executes in emission order, the next pair's
        # weight transposes and K/Q projection are chopped into small units
        # and paced evenly through this pair's attention emission, so the PE
        # always has an exp-independent matmul available where an AV or
        # scores matmul would otherwise wait on the scalar engine.
        def prep_pair_units(p):
            """DMA+cast now; PE transposes deferred as filler units."""
            wtK = wtq.tile([P, CG, P], bf16, tag="wtq")
            wnK = load_cast(wqkv_h[(8 + p) * P : (9 + p) * P, :], f"wk{p}")
            wtQ = wtq.tile([P, CG, P], bf16, tag="wtq")
            wnQ = load_cast(
                wqkv_h[p * P : (p + 1) * P, :], f"wq{p}", scale=float(SCALE)
            )
            units = []
            for wt, wn in ((wtK, wnK), (wtQ, wnQ)):
                for g4 in (0, 4):

                    def u(wt=wt, wn=wn, g4=g4):
                        psf = ps_pool.tile([P, QB], f32, tag="ps")
                        ps = psf[:, :].bitcast(bf16)[:, 0 : 4 * P]
                        for k in range(4):
                            nc.tensor.transpose(
                                ps[:, k * P : (k + 1) * P],
                                wn[:, (g4 + k) * P : (g4 + k + 1) * P],
                                ident,
                            )
                        nc.vector.tensor_copy(wt[:, g4 : g4 + 4, :], ps)

                    units.append(u)
            return wtK, wtQ, units

        def proj_pair_units(wtK, wtQ):
            """K/Q projection as filler units: each unit fully accumulates
            one [P, QB] psum tile (8 matmuls) and evacuates it to bf16."""
            KT = ktp.tile([P, N], bf16, tag="KT")
            QT = qtp.tile([P, TQ], bf16, tag="QT")
            units = []
            st = {}

            def half(wt, dst, col0, gh, key):
                # first half allocates the psum tile; second half evacuates
                if gh == 0:
                    st[key] = ps_pool.tile([P, QB], f32, tag="ps", name=key)
                ps = st[key]
                for g in range(gh, gh + 4):
                    nc.tensor.matmul(
                        ps,
                        wt[:, g, :],
                        xT[g][:, col0 : col0 + QB],
                        start=(g == 0),
                        stop=(g == CG - 1),
                    )
                if gh == 4:
                    nc.vector.tensor_copy(dst[:, col0 : col0 + QB], ps)

            for tb4 in range(4):
                for gh in (0, 4):
                    units.append(
                        lambda tb4=tb4, gh=gh: half(
                            wtK, KT, tb4 * QB, gh, "pjk"
                        )
                    )
            for k in range(2):
                for gh in (0, 4):
                    units.append(
                        lambda k=k, gh=gh: half(wtQ, QT, k * QB, gh, "pjq")
                    )
            return KT, QT, units

        bias = None
        wtK0, wtQ0, u0 = prep_pair_units(0)
        for u in u0:
            u()
        cur_kt, cur_qt, up0 = proj_pair_units(wtK0, wtQ0)
        for u in up0:
            u()
        for p in range(NPAIR):
            KT, QT = cur_kt, cur_qt
            pending = []
            if p + 1 < NPAIR:
                wtK, wtQ, prep_units = prep_pair_units(p + 1)
                cur_kt, cur_qt, proj_units = proj_pair_units(wtK, wtQ)
                pending = prep_units + proj_units
            else:
                # Pair 7 has no next-pair projection; use the out-projection
                # weight/bias prep as its attention filler instead.
                bias = big.tile([P, C], f32, name="bias")
                bl = const.tile([1, C], f32)
                nc.gpsimd.dma_start(bl, bout_h.unsqueeze(0))
                wo_lds = {0: w_chunk_loads(wout_h, 0)}
                wo_state = {}

                def bias_unit():
                    for hh in range(2):
                        ps = ps_pool.tile([P, QB], f32, tag="ps")
                        nc.tensor.matmul(
                            ps, onesrow, bl[0:1, hh * QB : (hh + 1) * QB]
                        )
                        nc.scalar.copy(bias[:, hh * QB : (hh + 1) * QB], ps)

                def wo_unit(which, g4):
                    if which not in wo_state:
                        wo_state[which] = wtv.tile(
                            [P, CG, 4 * P], bf16, tag="wtv",
                            name=f"woT{which}",
                        )
                    wt = wo_state[which]
                    ld = wo_lds[which]
                    for g in range(g4, g4 + 2):
                        psf = ps_pool.tile([P, QB], f32, tag="ps")
                        ps = psf[:, :].bitcast(bf16)[:, 0 : 4 * P]
                        for jj in range(4):
                            nc.tensor.transpose(
                                ps[:, jj * P : (jj + 1) * P],
                                ld[jj][:, g * P : (g + 1) * P],
                                ident,
                            )
                        nc.vector.tensor_copy(wt[:, g, :], ps)

                def wo_load1():
                    wo_lds[1] = w_chunk_loads(wout_h, 4)

                pending = [bias_unit]
                pending += [
                    lambda g4=g4: wo_unit(0, g4) for g4 in (0, 2, 4, 6)
                ]
                pending.append(wo_load1)
                pending += [
                    lambda g4=g4: wo_unit(1, g4) for g4 in (0, 2, 4, 6)
                ]

            nslots = NB * (TG // 2 + 2)
            nunits = len(pending)
            state = {"slot": 0, "popped": 0}

            def fill():
                state["slot"] += 1
                target = min(
                    nunits, -(-state["slot"] * nunits // nslots)
                )
                while state["popped"] < target:
                    pending[state["popped"]]()
                    state["popped"] += 1

            # ---- attention for head pair p ----
            # AV trails the scores/exp stream by 2 key-chunks so exp output
            # is ready when the AV matmul issues; one filler unit per chunk
            # soaks up the exp-rate deficit (exp 2218ns/chunk vs 1720ns of
            # scores+AV matmuls).
            NJ = TG // 2
            for tb in range(NB):
                qa = QT[0:64, tb * QB : (tb + 1) * QB]
                qb = QT[64:128, tb * QB : (tb + 1) * QB]
                pts = []
                ata = at_ps.tile([P, QB], f32, tag="at", name="ata")
                atb = at_ps.tile([P, QB], f32, tag="at", name="atb")

                def av(jj):
                    pta, ptb = pts[jj]
                    for k in range(2):
                        j = 2 * jj + k
                        first, last = (j == 0), (j == TG - 1)
                        nc.tensor.matmul(
                            ata,
                            V[j][:, 192 * p : 192 * p + 128],
                            pta[:, k * QB : (k + 1) * QB],
                            start=first,
                            stop=last,
                        )
                        nc.tensor.matmul(
                            atb,
                            V[j][:, 192 * p + 64 : 192 * p + 192],
                            ptb[:, k * QB : (k + 1) * QB],
                            start=first,
                            stop=last,
                        )

                for jj in range(NJ):
                    sta = st_ps.tile([P, 2 * QB], f32, tag="st", name="sta")
                    stb = st_ps.tile([P, 2 * QB], f32, tag="st", name="stb")
                    for k in range(2):
                        j = 2 * jj + k
                        nc.tensor.matmul(
                            sta[:, k * QB : (k + 1) * QB],
                            KT[0:64, j * P : (j + 1) * P],
                            qa,
                        )
                        nc.tensor.matmul(
                            stb[:, k * QB : (k + 1) * QB],
                            KT[64:128, j * P : (j + 1) * P],
                            qb,
                        )
                    pta = ptp.tile([P, 2 * QB], bf16, tag="pt", name="pta")
                    ptb = ptp.tile([P, 2 * QB], bf16, tag="pt", name="ptb")
                    nc.scalar.activation(pta, sta, AF.Exp)
                    nc.scalar.activation(ptb, stb, AF.Exp)
                    pts.append((pta, ptb))
                    if jj >= 2:
                        av(jj - 2)
                    fill()
                av(NJ - 2)
                fill()
                av(NJ - 1)
                fill()

                combA = recp.tile([64, QB], f32, tag="combA", bufs=1)
                nc.vector.tensor_copy(combA, ata[64:128, :])
                rtA = recp.tile([64, QB], f32, tag="rtA", bufs=1)
                nc.vector.reciprocal_approx_fast(rtA, combA)
                rtB = recp.tile([64, QB], f32, tag="rtB", bufs=1)
                nc.vector.reciprocal_approx_fast(rtB, atb[0:64, :])
                ao = aoT[p][:, tb * QB : (tb + 1) * QB]
                nc.vector.tensor_mul(ao[0:64, :], ata[0:64, :], rtA)
                nc.vector.tensor_mul(ao[64:128, :], atb[64:128, :], rtB)

            while state["popped"] < nunits:
                pending[state["popped"]]()
                state["popped"] += 1

        # ---------------- out projection ----------------
        for i in range(TQ // P):
            ob = outp.tile([P, C], f32, tag="ob")
            for oh, woT in ((0, wo_state[0]), (1, wo_state[1])):
                ps = ps_pool.tile([P, QB], f32, tag="ps")
                for g in range(CG):
                    nc.tensor.matmul(
                        ps,
                        aoT[g][:, i * P : (i + 1) * P],
                        woT[:, g, :],
                        start=(g == 0),
                        stop=(g == CG - 1),
                    )
                nc.vector.tensor_add(
                    ob[:, oh * QB : (oh + 1) * QB],
                    ps,
                    bias[:, oh * QB : (oh + 1) * QB],
                )
            nc.sync.dma_start(y_h[i * P : (i + 1) * P, :], ob)


def _run(in_maps, trace=False):
    from concourse.bass_utils import run_bass_kernel_spmd

    if "nc" not in _CACHE:
        _CACHE["nc"] = _build()
    nc = _CACHE["nc"]
    return run_bass_kernel_spmd(
        nc, in_maps, core_ids=list(range(8)), trace=trace
    )


def _make_in_maps(x, w_qkv, w_out, b_out):
    x = np.ascontiguousarray(np.asarray(x, dtype=np.float32))
    w_qkv = np.ascontiguousarray(np.asarray(w_qkv, dtype=np.float32))
    w_out = np.ascontiguousarray(np.asarray(w_out, dtype=np.float32))
    b_out = np.ascontiguousarray(np.asarray(b_out, dtype=np.float32))
    in_maps = []
    for c in range(8):
        b, h = divmod(c, 2)
        xb = x[b]
        if h:
            xb = np.ascontiguousarray(
                np.concatenate([xb[TQ:], xb[:TQ]], axis=0)
            )
        in_maps.append(
            {"x": xb, "w_qkv": w_qkv, "w_out": w_out, "b_out": b_out}
        )
    return in_maps


def _gather(results):
    y = np.empty((B, N, C), dtype=np.float32)
    for c in range(8):
        b, h = divmod(c, 2)
        y[b, h * TQ : (h + 1) * TQ, :] = results[c]["y"]
    return y


def kernel(x, w_qkv, w_out, b_out):
    res = _run(_make_in_maps(x, w_qkv, w_out, b_out), trace=False)
    return _gather(res.results)



# revision 23
# speedup vs baseline: 1.0003x; 1.0003x over previous
"""Multi-head attention (qkv proj + softmax attention + out proj) on 8 trn2 cores.

Sharding: zero-collective. Core c handles batch b=c//2 and query-half h=c%2
(1024 queries). The host passes x[b] ROTATED by h*1024 tokens so that each
core's queries are always local tokens 0..1023 while K/V still cover all 2048
tokens (key order is irrelevant to softmax). Host concatenates the 8 output
slabs [1024, 1024] into [4, 2048, 1024].

Per-core kernel (all on one NeuronCore, Tile-scheduled):
  1. PE-transpose x and the weights to contraction-major layout (fp32 in,
     bf16 out via the DVE PSUM-evacuation copy). x casts ride the (otherwise
     idle) ScalarE; V-projection weights load between the two x halves.
  2. QKV projection in bf16: QT/KT stored feature-major [d, t] so each
     128-row tile holds a PAIR of heads (2 x 64 d-rows); V stored token-major
     with interleaved ones columns (fused softmax-sum rows come out of the
     AV matmul for free).
  3. Attention per head-pair: QK^T into PSUM, softmax WITHOUT
     max-subtraction (scores are ~N(0,1); fp32 exp is safe), exp on ScalarE
     straight out of PSUM, AV+sums fused, fast approximate reciprocal on DVE.
  4. Out-projection in bf16 + bias, fp32 result.

Scheduling: engines execute in per-engine EMISSION order, so all overlap is
explicit software pipelining. The exp stream on ScalarE (2218ns per key
chunk) outpaces this pair's scores+AV matmuls (1720ns/chunk), so the NEXT
pair's weight transposes and K/Q projection are chopped into ~4-matmul units
and paced one-per-chunk through the attention emission; AV trails the
scores/exp stream by two chunks so probs are ready when AV issues. Pair 7
uses the out-projection weight/bias prep as its filler. This keeps the PE
gap-free through the whole mid-kernel (measured ~90% TensorE busy).
"""

import numpy as np

B, N, C = 4, 2048, 1024
H, D = 16, 64
P = 128
CG = C // P            # 8 contraction groups
TG = N // P            # 16 key-token chunks
TQ = N // 2            # 1024 queries per core
QB = 512               # query block (psum bank)
NB = TQ // QB          # 2
NPAIR = H // 2         # 8 head pairs
SCALE = 1.0 / np.sqrt(D).astype(np.float32)
DEBUG_DUMPS = False

_CACHE = {}


def _build():
    import concourse.tile as tile
    from concourse import bacc, mybir

    f32 = mybir.dt.float32
    nc = bacc.Bacc(
        "TRN2", target_bir_lowering=False, debug=False, num_devices=8
    )
    x_h = nc.dram_tensor("x", [N, C], f32, kind="ExternalInput").ap()
    wqkv_h = nc.dram_tensor("w_qkv", [3 * C, C], f32, kind="ExternalInput").ap()
    wout_h = nc.dram_tensor("w_out", [C, C], f32, kind="ExternalInput").ap()
    bout_h = nc.dram_tensor("b_out", [C], f32, kind="ExternalInput").ap()
    y_h = nc.dram_tensor("y", [TQ, C], f32, kind="ExternalOutput").ap()

    with tile.TileContext(nc) as tc:
        _emit(tc, x_h, wqkv_h, wout_h, bout_h, y_h)
    nc.compile()
    return nc


def _emit(tc, x_h, wqkv_h, wout_h, bout_h, y_h):
    from contextlib import ExitStack

    from concourse import mybir
    from concourse.masks import make_identity

    f32 = mybir.dt.float32
    bf16 = mybir.dt.bfloat16
    AF = mybir.ActivationFunctionType
    nc = tc.nc

    with ExitStack() as ctx:
        # ---------------- pools ----------------
        const = ctx.enter_context(tc.tile_pool(name="const", bufs=1))
        big = ctx.enter_context(tc.tile_pool(name="big", bufs=1))
        land = ctx.enter_context(tc.tile_pool(name="land", bufs=3))
        cstp = ctx.enter_context(tc.tile_pool(name="cstp", bufs=7))
        ktp = ctx.enter_context(tc.tile_pool(name="ktp", bufs=2))
        qtp = ctx.enter_context(tc.tile_pool(name="qtp", bufs=2))
        wtq = ctx.enter_context(tc.tile_pool(name="wtq", bufs=4))
        wtv = ctx.enter_context(tc.tile_pool(name="wtv", bufs=2))
        ptp = ctx.enter_context(tc.tile_pool(name="ptp", bufs=13))
        recp = ctx.enter_context(tc.tile_pool(name="recp", bufs=2))
        outp = ctx.enter_context(tc.tile_pool(name="outp", bufs=2))
        ps_pool = ctx.enter_context(
            tc.tile_pool(name="ps_pool", bufs=2, space="PSUM")
        )
        st_ps = ctx.enter_context(
            tc.tile_pool(name="st_ps", bufs=2, space="PSUM")
        )
        at_ps = ctx.enter_context(
            tc.tile_pool(name="at_ps", bufs=2, space="PSUM")
        )

        # ---------------- constants ----------------
        ident = const.tile([P, P], bf16)
        make_identity(nc, ident)
        onesrow = const.tile([1, P], f32)
        nc.gpsimd.memset(onesrow, 1.0)

        # ---------------- persistent tensors ----------------
        # V layout per pair p: cols [192p,192p+64)=V_h2p, [192p+64,+128)=ones,
        # [192p+128,+192)=V_h2p+1. The AV stationary operand for head A is
        # cols [192p,192p+128) = [V_A | ones] (output rows 0:64 = attention
        # out, rows 64:128 = softmax sums, replicated); for head B it is cols
        # [192p+64,192p+192) = [ones | V_B] (sums on rows 0:64, attention out
        # on rows 64:128). The ones block is shared between the two heads.
        xT = [big.tile([P, N], bf16, name=f"xT{g}") for g in range(CG)]
        V = [big.tile([P, 192 * NPAIR], bf16, name=f"V{i}") for i in range(TG)]
        aoT = [big.tile([P, TQ], bf16, name=f"aoT{g}") for g in range(CG)]
        for i in range(TG):
            v3i = V[i].rearrange("p (q e) -> p q e", e=64)
            nc.vector.memset(v3i[:, 1 : 3 * NPAIR : 3], 1.0)

        # -------- load x (fp32), cast bf16, PE-transpose into xT --------
        def load_cast(src_ap, name, scale=None, on_scalar=False):
            t = land.tile([P, C], f32, tag="land", name=f"ld{name}")
            nc.sync.dma_start(t, src_ap)
            b = cstp.tile([P, C], bf16, tag="cst", name=f"cs{name}")
            if on_scalar:
                if scale is None:
                    nc.scalar.copy(b, t)
                else:
                    nc.scalar.mul(b, t, scale)
            elif scale is None:
                nc.vector.tensor_copy(b, t)
            else:
                nc.vector.tensor_scalar_mul(b, t, scale)
            return b

        def w_chunk_loads(src_h, row0):
            return [
                load_cast(
                    src_h[(row0 + jj) * P : (row0 + jj + 1) * P, :],
                    f"w{row0}_{jj}",
                )
                for jj in range(4)
            ]

        def w_chunk_transpose(wn, pool, tag):
            wt = pool.tile([P, CG, 4 * P], bf16, tag=tag)
            for g in range(CG):
                psf = ps_pool.tile([P, QB], f32, tag="ps")
                ps = psf[:, :].bitcast(bf16)[:, 0 : 4 * P]
                for jj in range(4):
                    nc.tensor.transpose(
                        ps[:, jj * P : (jj + 1) * P],
                        wn[jj][:, g * P : (g + 1) * P],
                        ident,
                    )
                nc.vector.tensor_copy(wt[:, g, :], ps)
            return wt

        def w_chunk_T(src_h, row0, pool, tag):
            """Load+transpose 4 consecutive 128-row chunks of a [*, C] fp32
            weight into a [P, CG, 512] bf16 tile (contraction-major)."""
            return w_chunk_transpose(w_chunk_loads(src_h, row0), pool, tag)

        # x casts go on the scalar engine: it is idle until the first exp,
        # which parallelizes the cast chain with the psum-evac copies on DVE.
        # The V-projection weight DMAs are interleaved between the two x
        # halves so their prep overlaps the x-load tail instead of queueing
        # behind all 8.4MB of x.
        xb16 = [
            load_cast(x_h[i * P : (i + 1) * P, :], f"x{i}", on_scalar=True)
            for i in range(8)
        ]
        wv0_ld = w_chunk_loads(wqkv_h, 16)
        xb16 += [
            load_cast(x_h[i * P : (i + 1) * P, :], f"x{i}", on_scalar=True)
            for i in range(8, TG)
        ]

        # [128,128] bf16 PE transposes batched into one psum bank, one copy.
        # Smaller first batches so the PE starts as soon as chunk 0 lands.
        def xpose(i4, nb):
            for g in range(CG):
                psf = ps_pool.tile([P, QB], f32, tag="ps")
                ps = psf[:, :].bitcast(bf16)[:, 0 : nb * P]
                for k in range(nb):
                    nc.tensor.transpose(
                        ps[:, k * P : (k + 1) * P],
                        xb16[i4 + k][:, g * P : (g + 1) * P],
                        ident,
                    )
                nc.vector.tensor_copy(xT[g][:, i4 * P : (i4 + nb) * P], ps)

        # ---------------- V projection (interleaved with x prep) -----------
        def v_proj(fh, wvT, i0, i1):
            for i in range(i0, i1):
                ps = ps_pool.tile([P, QB], f32, tag="ps")
                for g in range(CG):
                    nc.tensor.matmul(
                        ps,
                        xT[g][:, i * P : (i + 1) * P],
                        wvT[:, g, :],
                        start=(g == 0),
                        stop=(g == CG - 1),
                    )
                # psum cols = v features [512*fh, 512*(fh+1)) = heads
                # 8fh..8fh+7. Scatter per-head 64-col blocks into the
                # interleaved V layout: head h -> col 192*(h//2)+128*(h%2).
                ps3 = ps.rearrange("p (k e) -> p k e", e=64)
                v3 = V[i].rearrange("p (q e) -> p q e", e=64)
                b0 = 12 * fh
                nc.vector.tensor_copy(v3[:, b0 : b0 + 12 : 3], ps3[:, 0::2])
                nc.vector.tensor_copy(
                    v3[:, b0 + 2 : b0 + 12 : 3], ps3[:, 1::2]
                )

        for i4, nb in ((0, 1), (1, 1), (2, 2), (4, 4)):
            xpose(i4, nb)
        wvT0 = w_chunk_transpose(wv0_ld, wtv, "wtv")
        wv1_ld = w_chunk_loads(wqkv_h, 20)
        v_proj(0, wvT0, 0, 8)
        xpose(8, 4)
        xpose(12, 4)
        wvT1 = w_chunk_transpose(wv1_ld, wtv, "wtv")
        v_proj(0, wvT0, 8, TG)
        v_proj(1, wvT1, 0, TG)

        # ---------------- per-pair: K proj, Q proj, attention ----------------
        # The exp on ScalarE is the rate limiter inside an attention block
        # (17.7us of exp vs 10.2us of scores+AV matmuls per (pair, tb)).
        # Since each engine executes in emission order, the next pair's
        # weight transposes and K/Q projection are chopped into small units
        # and paced evenly through this pair's attention emission, so the PE
        # always has an exp-independent matmul available where an AV or
        # scores matmul would otherwise wait on the scalar engine.
        def prep_pair_units(p):
            """DMA+cast now; PE transposes deferred as filler units."""
            wtK = wtq.tile([P, CG, P], bf16, tag="wtq")
            wnK = load_cast(wqkv_h[(8 + p) * P : (9 + p) * P, :], f"wk{p}")
            wtQ = wtq.tile([P, CG, P], bf16, tag="wtq")
            wnQ = load_cast(
                wqkv_h[p * P : (p + 1) * P, :], f"wq{p}", scale=float(SCALE)
            )
            units = []
            for wt, wn in ((wtK, wnK), (wtQ, wnQ)):
                for g4 in (0, 4):

                    def u(wt=wt, wn=wn, g4=g4):
                        psf = ps_pool.tile([P, QB], f32, tag="ps")
                        ps = psf[:, :].bitcast(bf16)[:, 0 : 4 * P]
                        for k in range(4):
                            nc.tensor.transpose(
                                ps[:, k * P : (k + 1) * P],
                                wn[:, (g4 + k) * P : (g4 + k + 1) * P],
                                ident,
                            )
                        nc.vector.tensor_copy(wt[:, g4 : g4 + 4, :], ps)

                    units.append(u)
            return wtK, wtQ, units

        def proj_pair_units(wtK, wtQ):
            """K/Q projection as filler units: each unit fully accumulates
            one [P, QB] psum tile (8 matmuls) and evacuates it to bf16."""
            KT = ktp.tile([P, N], bf16, tag="KT")
            QT = qtp.tile([P, TQ], bf16, tag="QT")
            units = []
            st = {}

            def half(wt, dst, col0, gh, key):
                # first half allocates the psum tile; second half evacuates
                if gh == 0:
                    st[key] = ps_pool.tile([P, QB], f32, tag="ps", name=key)
                ps = st[key]
                for g in range(gh, gh + 4):
                    nc.tensor.matmul(
                        ps,
                        wt[:, g, :],
                        xT[g][:, col0 : col0 + QB],
                        start=(g == 0),
                        stop=(g == CG - 1),
                    )
                if gh == 4:
                    nc.vector.tensor_copy(dst[:, col0 : col0 + QB], ps)

            for tb4 in range(4):
                for gh in (0, 4):
                    units.append(
                        lambda tb4=tb4, gh=gh: half(
                            wtK, KT, tb4 * QB, gh, "pjk"
                        )
                    )
            for k in range(2):
                for gh in (0, 4):
                    units.append(
                        lambda k=k, gh=gh: half(wtQ, QT, k * QB, gh, "pjq")
                    )
            return KT, QT, units

        bias = None
        wtK0, wtQ0, u0 = prep_pair_units(0)
        for u in u0:
            u()
        cur_kt, cur_qt, up0 = proj_pair_units(wtK0, wtQ0)
        for u in up0:
            u()
        for p in range(NPAIR):
            KT, QT = cur_kt, cur_qt
            pending = []
            if p + 1 < NPAIR:
                wtK, wtQ, prep_units = prep_pair_units(p + 1)
                cur_kt, cur_qt, proj_units = proj_pair_units(wtK, wtQ)
                pending = prep_units + proj_units
            else:
                # Pair 7 has no next-pair projection; use the out-projection
                # weight/bias prep as its attention filler instead.
                bias = big.tile([P, C], f32, name="bias")
                bl = const.tile([1, C], f32)
                nc.gpsimd.dma_start(bl, bout_h.unsqueeze(0))
                wo_lds = {0: w_chunk_loads(wout_h, 0)}
                wo_state = {}

                def bias_unit():
                    for hh in range(2):
                        ps = ps_pool.tile([P, QB], f32, tag="ps")
                        nc.tensor.matmul(
                            ps, onesrow, bl[0:1, hh * QB : (hh + 1) * QB]
                        )
                        nc.scalar.copy(bias[:, hh * QB : (hh + 1) * QB], ps)

                def wo_unit(which, g4):
                    if which not in wo_state:
                        wo_state[which] = wtv.tile(
                            [P, CG, 4 * P], bf16, tag="wtv",
                            name=f"woT{which}",
                        )
                    wt = wo_state[which]
                    ld = wo_lds[which]
                    for g in range(g4, g4 + 2):
                        psf = ps_pool.tile([P, QB], f32, tag="ps")
                        ps = psf[:, :].bitcast(bf16)[:, 0 : 4 * P]
                        for jj in range(4):
                            nc.tensor.transpose(
                                ps[:, jj * P : (jj + 1) * P],
                                ld[jj][:, g * P : (g + 1) * P],
                                ident,
                            )
                        nc.vector.tensor_copy(wt[:, g, :], ps)

                def wo_load1():
                    wo_lds[1] = w_chunk_loads(wout_h, 4)

                ob_state = {}

                def op_unit(i, oh):
                    if oh == 0:
                        ob_state[i] = outp.tile(
                            [P, C], f32, tag="ob", name=f"ob{i}"
                        )
                    ob = ob_state[i]
                    woT = wo_state[oh]
                    ps = ps_pool.tile([P, QB], f32, tag="ps")
                    for g in range(CG):
                        nc.tensor.matmul(
                            ps,
                            aoT[g][:, i * P : (i + 1) * P],
                            woT[:, g, :],
                            start=(g == 0),
                            stop=(g == CG - 1),
                        )
                    nc.vector.tensor_add(
                        ob[:, oh * QB : (oh + 1) * QB],
                        ps,
                        bias[:, oh * QB : (oh + 1) * QB],
                    )
                    if oh == 1:
                        nc.sync.dma_start(y_h[i * P : (i + 1) * P, :], ob)

                pending = [bias_unit]
                pending += [
                    lambda g4=g4: wo_unit(0, g4) for g4 in (0, 2, 4, 6)
                ]
                pending.append(wo_load1)
                pending += [
                    lambda g4=g4: wo_unit(1, g4) for g4 in (0, 2, 4, 6)
                ]

            nslots = NB * (TG // 2 + 2)
            state = {"slot": 0, "popped": 0}

            def fill():
                state["slot"] += 1
                n = len(pending)
                target = min(n, -(-state["slot"] * n // nslots))
                while state["popped"] < target:
                    pending[state["popped"]]()
                    state["popped"] += 1

            # ---- attention for head pair p ----
            # AV trails the scores/exp stream by 2 key-chunks so exp output
            # is ready when the AV matmul issues; one filler unit per chunk
            # soaks up the exp-rate deficit (exp 2218ns/chunk vs 1720ns of
            # scores+AV matmuls).
            NJ = TG // 2
            for tb in range(NB):
                qa = QT[0:64, tb * QB : (tb + 1) * QB]
                qb = QT[64:128, tb * QB : (tb + 1) * QB]
                pts = []
                ata = at_ps.tile([P, QB], f32, tag="at", name="ata")
                atb = at_ps.tile([P, QB], f32, tag="at", name="atb")

                def av(jj):
                    pta, ptb = pts[jj]
                    for k in range(2):
                        j = 2 * jj + k
                        first, last = (j == 0), (j == TG - 1)
                        nc.tensor.matmul(
                            ata,
                            V[j][:, 192 * p : 192 * p + 128],
                            pta[:, k * QB : (k + 1) * QB],
                            start=first,
                            stop=last,
                        )
                        nc.tensor.matmul(
                            atb,
                            V[j][:, 192 * p + 64 : 192 * p + 192],
                            ptb[:, k * QB : (k + 1) * QB],
                            start=first,
                            stop=last,
                        )

                for jj in range(NJ):
                    sta = st_ps.tile([P, 2 * QB], f32, tag="st", name="sta")
                    stb = st_ps.tile([P, 2 * QB], f32, tag="st", name="stb")
                    for k in range(2):
                        j = 2 * jj + k
                        nc.tensor.matmul(
                            sta[:, k * QB : (k + 1) * QB],
                            KT[0:64, j * P : (j + 1) * P],
                            qa,
                        )
                        nc.tensor.matmul(
                            stb[:, k * QB : (k + 1) * QB],
                            KT[64:128, j * P : (j + 1) * P],
                            qb,
                        )
                    pta = ptp.tile([P, 2 * QB], bf16, tag="pt", name="pta")
                    ptb = ptp.tile([P, 2 * QB], bf16, tag="pt", name="ptb")
                    nc.scalar.activation(pta, sta, AF.Exp)
                    nc.scalar.activation(ptb, stb, AF.Exp)
                    pts.append((pta, ptb))
                    if jj >= 2:
                        av(jj - 2)
                    fill()
                av(NJ - 2)
                fill()
                av(NJ - 1)
                fill()

                combA = recp.tile([64, QB], f32, tag="combA", bufs=1)
                nc.vector.tensor_copy(combA, ata[64:128, :])
                rtA = recp.tile([64, QB], f32, tag="rtA", bufs=1)
                nc.vector.reciprocal_approx_fast(rtA, combA)
                rtB = recp.tile([64, QB], f32, tag="rtB", bufs=1)
                nc.vector.reciprocal_approx_fast(rtB, atb[0:64, :])
                ao = aoT[p][:, tb * QB : (tb + 1) * QB]
                nc.vector.tensor_mul(ao[0:64, :], ata[0:64, :], rtA)
                nc.vector.tensor_mul(ao[64:128, :], atb[64:128, :], rtB)

                if p == NPAIR - 1:
                    # This tb's aoT[7] columns are final now: the out
                    # projection for its 4 token blocks joins the filler
                    # stream (tb0's blocks overlap tb1's attention; tb1's
                    # drain right after).
                    for i in range(tb * 4, tb * 4 + 4):
                        for oh in range(2):
                            pending.append(
                                lambda i=i, oh=oh: op_unit(i, oh)
                            )

            while state["popped"] < len(pending):
                pending[state["popped"]]()
                state["popped"] += 1


def _run(in_maps, trace=False):
    from concourse.bass_utils import run_bass_kernel_spmd

    if "nc" not in _CACHE:
        _CACHE["nc"] = _build()
    nc = _CACHE["nc"]
    return run_bass_kernel_spmd(
        nc, in_maps, core_ids=list(range(8)), trace=trace
    )


def _make_in_maps(x, w_qkv, w_out, b_out):
    x = np.ascontiguousarray(np.asarray(x, dtype=np.float32))
    w_qkv = np.ascontiguousarray(np.asarray(w_qkv, dtype=np.float32))
    w_out = np.ascontiguousarray(np.asarray(w_out, dtype=np.float32))
    b_out = np.ascontiguousarray(np.asarray(b_out, dtype=np.float32))
    in_maps = []
    for c in range(8):
        b, h = divmod(c, 2)
        xb = x[b]
        if h:
            xb = np.ascontiguousarray(
                np.concatenate([xb[TQ:], xb[:TQ]], axis=0)
            )
        in_maps.append(
            {"x": xb, "w_qkv": w_qkv, "w_out": w_out, "b_out": b_out}
        )
    return in_maps


def _gather(results):
    y = np.empty((B, N, C), dtype=np.float32)
    for c in range(8):
        b, h = divmod(c, 2)
        y[b, h * TQ : (h + 1) * TQ, :] = results[c]["y"]
    return y


def kernel(x, w_qkv, w_out, b_out):
    res = _run(_make_in_maps(x, w_qkv, w_out, b_out), trace=False)
    return _gather(res.results)



# revision 25
# speedup vs baseline: 1.0032x; 1.0029x over previous
"""Multi-head attention (qkv proj + softmax attention + out proj) on 8 trn2 cores.

Sharding: zero-collective. Core c handles batch b=c//2 and query-half h=c%2
(1024 queries). The host passes x[b] ROTATED by h*1024 tokens so that each
core's queries are always local tokens 0..1023 while K/V still cover all 2048
tokens (key order is irrelevant to softmax). Host concatenates the 8 output
slabs [1024, 1024] into [4, 2048, 1024].

Per-core kernel (all on one NeuronCore, Tile-scheduled):
  1. PE-transpose x and the weights to contraction-major layout (fp32 in,
     bf16 out via the DVE PSUM-evacuation copy). x casts ride the (otherwise
     idle) ScalarE; V-projection weights load between the two x halves.
  2. QKV projection in bf16: QT/KT stored feature-major [d, t] so each
     128-row tile holds a PAIR of heads (2 x 64 d-rows); V stored token-major
     with interleaved ones columns (fused softmax-sum rows come out of the
     AV matmul for free).
  3. Attention per head-pair: QK^T into PSUM, softmax WITHOUT
     max-subtraction (scores are ~N(0,1); fp32 exp is safe), exp on ScalarE
     straight out of PSUM, AV+sums fused, fast approximate reciprocal on DVE.
  4. Out-projection in bf16 + bias, fp32 result.

Scheduling: engines execute in per-engine EMISSION order, so all overlap is
explicit software pipelining. The exp stream on ScalarE (2218ns per key
chunk) outpaces this pair's scores+AV matmuls (1720ns/chunk), so the NEXT
pair's weight transposes and K/Q projection are chopped into ~4-matmul units
and paced one-per-chunk through the attention emission; AV trails the
scores/exp stream by two chunks so probs are ready when AV issues. Pair 7
uses the out-projection weight/bias prep as its filler. This keeps the PE
gap-free through the whole mid-kernel (measured ~90% TensorE busy).
"""

import numpy as np

B, N, C = 4, 2048, 1024
H, D = 16, 64
P = 128
CG = C // P            # 8 contraction groups
TG = N // P            # 16 key-token chunks
TQ = N // 2            # 1024 queries per core
QB = 512               # query block (psum bank)
NB = TQ // QB          # 2
NPAIR = H // 2         # 8 head pairs
SCALE = 1.0 / np.sqrt(D).astype(np.float32)
DEBUG_DUMPS = False

_CACHE = {}


def _build():
    import concourse.tile as tile
    from concourse import bacc, mybir

    f32 = mybir.dt.float32
    nc = bacc.Bacc(
        "TRN2", target_bir_lowering=False, debug=False, num_devices=8
    )
    x_h = nc.dram_tensor("x", [N, C], f32, kind="ExternalInput").ap()
    wqkv_h = nc.dram_tensor("w_qkv", [3 * C, C], f32, kind="ExternalInput").ap()
    wout_h = nc.dram_tensor("w_out", [C, C], f32, kind="ExternalInput").ap()
    bout_h = nc.dram_tensor("b_out", [C], f32, kind="ExternalInput").ap()
    y_h = nc.dram_tensor("y", [TQ, C], f32, kind="ExternalOutput").ap()

    with tile.TileContext(nc) as tc:
        _emit(tc, x_h, wqkv_h, wout_h, bout_h, y_h)
    nc.compile()
    return nc


def _emit(tc, x_h, wqkv_h, wout_h, bout_h, y_h):
    from contextlib import ExitStack

    from concourse import mybir
    from concourse.masks import make_identity

    f32 = mybir.dt.float32
    bf16 = mybir.dt.bfloat16
    AF = mybir.ActivationFunctionType
    nc = tc.nc

    with ExitStack() as ctx:
        # ---------------- pools ----------------
        const = ctx.enter_context(tc.tile_pool(name="const", bufs=1))
        big = ctx.enter_context(tc.tile_pool(name="big", bufs=1))
        land = ctx.enter_context(tc.tile_pool(name="land", bufs=3))
        cstp = ctx.enter_context(tc.tile_pool(name="cstp", bufs=7))
        ktp = ctx.enter_context(tc.tile_pool(name="ktp", bufs=2))
        qtp = ctx.enter_context(tc.tile_pool(name="qtp", bufs=2))
        wtq = ctx.enter_context(tc.tile_pool(name="wtq", bufs=4))
        wtv = ctx.enter_context(tc.tile_pool(name="wtv", bufs=2))
        ptp = ctx.enter_context(tc.tile_pool(name="ptp", bufs=13))
        recp = ctx.enter_context(tc.tile_pool(name="recp", bufs=2))
        outp = ctx.enter_context(tc.tile_pool(name="outp", bufs=2))
        ps_pool = ctx.enter_context(
            tc.tile_pool(name="ps_pool", bufs=2, space="PSUM")
        )
        st_ps = ctx.enter_context(
            tc.tile_pool(name="st_ps", bufs=2, space="PSUM")
        )
        at_ps = ctx.enter_context(
            tc.tile_pool(name="at_ps", bufs=2, space="PSUM")
        )

        # ---------------- constants ----------------
        ident = const.tile([P, P], bf16)
        make_identity(nc, ident)
        onesrow = const.tile([1, P], f32)
        nc.gpsimd.memset(onesrow, 1.0)

        # ---------------- persistent tensors ----------------
        # V layout per pair p: cols [192p,192p+64)=V_h2p, [192p+64,+128)=ones,
        # [192p+128,+192)=V_h2p+1. The AV stationary operand for head A is
        # cols [192p,192p+128) = [V_A | ones] (output rows 0:64 = attention
        # out, rows 64:128 = softmax sums, replicated); for head B it is cols
        # [192p+64,192p+192) = [ones | V_B] (sums on rows 0:64, attention out
        # on rows 64:128). The ones block is shared between the two heads.
        xT = [big.tile([P, N], bf16, name=f"xT{g}") for g in range(CG)]
        V = [big.tile([P, 192 * NPAIR], bf16, name=f"V{i}") for i in range(TG)]
        aoT = [big.tile([P, TQ], bf16, name=f"aoT{g}") for g in range(CG)]
        for i in range(TG):
            v3i = V[i].rearrange("p (q e) -> p q e", e=64)
            nc.vector.memset(v3i[:, 1 : 3 * NPAIR : 3], 1.0)

        # -------- load x (fp32), cast bf16, PE-transpose into xT --------
        def load_cast(src_ap, name, scale=None, on_scalar=False, on_gp=False):
            t = land.tile([P, C], f32, tag="land", name=f"ld{name}")
            nc.sync.dma_start(t, src_ap)
            b = cstp.tile([P, C], bf16, tag="cst", name=f"cs{name}")
            if on_scalar:
                if scale is None:
                    nc.scalar.copy(b, t)
                else:
                    nc.scalar.mul(b, t, scale)
            elif on_gp:
                nc.gpsimd.tensor_copy(b, t)
            elif scale is None:
                nc.vector.tensor_copy(b, t)
            else:
                nc.vector.tensor_scalar_mul(b, t, scale)
            return b

        def w_chunk_loads(src_h, row0):
            return [
                load_cast(
                    src_h[(row0 + jj) * P : (row0 + jj + 1) * P, :],
                    f"w{row0}_{jj}",
                )
                for jj in range(4)
            ]

        def w_chunk_transpose(wn, pool, tag):
            wt = pool.tile([P, CG, 4 * P], bf16, tag=tag)
            for g in range(CG):
                psf = ps_pool.tile([P, QB], f32, tag="ps")
                ps = psf[:, :].bitcast(bf16)[:, 0 : 4 * P]
                for jj in range(4):
                    nc.tensor.transpose(
                        ps[:, jj * P : (jj + 1) * P],
                        wn[jj][:, g * P : (g + 1) * P],
                        ident,
                    )
                nc.vector.tensor_copy(wt[:, g, :], ps)
            return wt

        def w_chunk_T(src_h, row0, pool, tag):
            """Load+transpose 4 consecutive 128-row chunks of a [*, C] fp32
            weight into a [P, CG, 512] bf16 tile (contraction-major)."""
            return w_chunk_transpose(w_chunk_loads(src_h, row0), pool, tag)

        # x casts go on the scalar engine: it is idle until the first exp,
        # which parallelizes the cast chain with the psum-evac copies on DVE.
        # The V-projection weight DMAs are interleaved between the two x
        # halves so their prep overlaps the x-load tail instead of queueing
        # behind all 8.4MB of x.
        xb16 = [
            load_cast(x_h[i * P : (i + 1) * P, :], f"x{i}", on_scalar=True)
            for i in range(8)
        ]
        wv0_ld = w_chunk_loads(wqkv_h, 16)
        xb16 += [
            load_cast(x_h[i * P : (i + 1) * P, :], f"x{i}", on_scalar=True)
            for i in range(8, TG)
        ]

        # [128,128] bf16 PE transposes batched into one psum bank, one copy.
        # Smaller first batches so the PE starts as soon as chunk 0 lands.
        def xpose(i4, nb):
            for g in range(CG):
                psf = ps_pool.tile([P, QB], f32, tag="ps")
                ps = psf[:, :].bitcast(bf16)[:, 0 : nb * P]
                for k in range(nb):
                    nc.tensor.transpose(
                        ps[:, k * P : (k + 1) * P],
                        xb16[i4 + k][:, g * P : (g + 1) * P],
                        ident,
                    )
                nc.vector.tensor_copy(xT[g][:, i4 * P : (i4 + nb) * P], ps)

        # ---------------- V projection (interleaved with x prep) -----------
        def v_proj(fh, wvT, i0, i1):
            for i in range(i0, i1):
                ps = ps_pool.tile([P, QB], f32, tag="ps")
                for g in range(CG):
                    nc.tensor.matmul(
                        ps,
                        xT[g][:, i * P : (i + 1) * P],
                        wvT[:, g, :],
                        start=(g == 0),
                        stop=(g == CG - 1),
                    )
                # psum cols = v features [512*fh, 512*(fh+1)) = heads
                # 8fh..8fh+7. Scatter per-head 64-col blocks into the
                # interleaved V layout: head h -> col 192*(h//2)+128*(h%2).
                ps3 = ps.rearrange("p (k e) -> p k e", e=64)
                v3 = V[i].rearrange("p (q e) -> p q e", e=64)
                b0 = 12 * fh
                nc.vector.tensor_copy(v3[:, b0 : b0 + 12 : 3], ps3[:, 0::2])
                nc.vector.tensor_copy(
                    v3[:, b0 + 2 : b0 + 12 : 3], ps3[:, 1::2]
                )

        for i4, nb in ((0, 1), (1, 1), (2, 2), (4, 4)):
            xpose(i4, nb)
        wvT0 = w_chunk_transpose(wv0_ld, wtv, "wtv")
        wv1_ld = w_chunk_loads(wqkv_h, 20)
        v_proj(0, wvT0, 0, 8)
        xpose(8, 4)
        xpose(12, 4)
        wvT1 = w_chunk_transpose(wv1_ld, wtv, "wtv")
        v_proj(0, wvT0, 8, TG)
        v_proj(1, wvT1, 0, TG)

        # ---------------- per-pair: K proj, Q proj, attention ----------------
        # The exp on ScalarE is the rate limiter inside an attention block
        # (17.7us of exp vs 10.2us of scores+AV matmuls per (pair, tb)).
        # Since each engine executes in emission order, the next pair's
        # weight transposes and K/Q projection are chopped into small units
        # and paced evenly through this pair's attention emission, so the PE
        # always has an exp-independent matmul available where an AV or
        # scores matmul would otherwise wait on the scalar engine.
        def prep_pair_units(p):
            """DMA+cast now; PE transposes deferred as filler units."""
            wtK = wtq.tile([P, CG, P], bf16, tag="wtq")
            wnK = load_cast(wqkv_h[(8 + p) * P : (9 + p) * P, :], f"wk{p}")
            wtQ = wtq.tile([P, CG, P], bf16, tag="wtq")
            wnQ = load_cast(
                wqkv_h[p * P : (p + 1) * P, :], f"wq{p}", scale=float(SCALE)
            )
            units = []
            for wt, wn in ((wtK, wnK), (wtQ, wnQ)):
                for g4 in (0, 4):

                    def u(wt=wt, wn=wn, g4=g4):
                        psf = ps_pool.tile([P, QB], f32, tag="ps")
                        ps = psf[:, :].bitcast(bf16)[:, 0 : 4 * P]
                        for k in range(4):
                            nc.tensor.transpose(
                                ps[:, k * P : (k + 1) * P],
                                wn[:, (g4 + k) * P : (g4 + k + 1) * P],
                                ident,
                            )
                        nc.vector.tensor_copy(wt[:, g4 : g4 + 4, :], ps)

                    units.append(u)
            return wtK, wtQ, units

        def proj_pair_units(wtK, wtQ):
            """K/Q projection as filler units: each unit fully accumulates
            one [P, QB] psum tile (8 matmuls) and evacuates it to bf16."""
            KT = ktp.tile([P, N], bf16, tag="KT")
            QT = qtp.tile([P, TQ], bf16, tag="QT")
            units = []
            st = {}

            def half(wt, dst, col0, gh, key):
                # first half allocates the psum tile; second half evacuates
                if gh == 0:
                    st[key] = ps_pool.tile([P, QB], f32, tag="ps", name=key)
                ps = st[key]
                for g in range(gh, gh + 4):
                    nc.tensor.matmul(
                        ps,
                        wt[:, g, :],
                        xT[g][:, col0 : col0 + QB],
                        start=(g == 0),
                        stop=(g == CG - 1),
                    )
                if gh == 4:
                    nc.vector.tensor_copy(dst[:, col0 : col0 + QB], ps)

            for tb4 in range(4):
                for gh in (0, 4):
                    units.append(
                        lambda tb4=tb4, gh=gh: half(
                            wtK, KT, tb4 * QB, gh, "pjk"
                        )
                    )
            for k in range(2):
                for gh in (0, 4):
                    units.append(
                        lambda k=k, gh=gh: half(wtQ, QT, k * QB, gh, "pjq")
                    )
            return KT, QT, units

        bias = None
        wtK0, wtQ0, u0 = prep_pair_units(0)
        for u in u0:
            u()
        cur_kt, cur_qt, up0 = proj_pair_units(wtK0, wtQ0)
        for u in up0:
            u()
        for p in range(NPAIR):
            KT, QT = cur_kt, cur_qt
            pending = []
            if p + 1 < NPAIR:
                wtK, wtQ, prep_units = prep_pair_units(p + 1)
                cur_kt, cur_qt, proj_units = proj_pair_units(wtK, wtQ)
                pending = prep_units + proj_units
            else:
                # Pair 7 has no next-pair projection; use the out-projection
                # weight/bias prep as its attention filler instead.
                bias = big.tile([P, C], f32, name="bias")
                bl = const.tile([1, C], f32)
                nc.gpsimd.dma_start(bl, bout_h.unsqueeze(0))
                wo_lds = {0: w_chunk_loads(wout_h, 0)}
                wo_state = {}

                def bias_unit():
                    for hh in range(2):
                        ps = ps_pool.tile([P, QB], f32, tag="ps")
                        nc.tensor.matmul(
                            ps, onesrow, bl[0:1, hh * QB : (hh + 1) * QB]
                        )
                        nc.scalar.copy(bias[:, hh * QB : (hh + 1) * QB], ps)

                def wo_unit(which, g4):
                    if which not in wo_state:
                        wo_state[which] = wtv.tile(
                            [P, CG, 4 * P], bf16, tag="wtv",
                            name=f"woT{which}",
                        )
                    wt = wo_state[which]
                    ld = wo_lds[which]
                    for g in range(g4, g4 + 2):
                        psf = ps_pool.tile([P, QB], f32, tag="ps")
                        ps = psf[:, :].bitcast(bf16)[:, 0 : 4 * P]
                        for jj in range(4):
                            nc.tensor.transpose(
                                ps[:, jj * P : (jj + 1) * P],
                                ld[jj][:, g * P : (g + 1) * P],
                                ident,
                            )
                        nc.vector.tensor_copy(wt[:, g, :], ps)

                def wo_load1():
                    wo_lds[1] = w_chunk_loads(wout_h, 4)

                ob_state = {}

                def op_unit(i, oh):
                    if oh == 0:
                        ob_state[i] = outp.tile(
                            [P, C], f32, tag="ob", name=f"ob{i}"
                        )
                    ob = ob_state[i]
                    woT = wo_state[oh]
                    ps = ps_pool.tile([P, QB], f32, tag="ps")
                    for g in range(CG):
                        nc.tensor.matmul(
                            ps,
                            aoT[g][:, i * P : (i + 1) * P],
                            woT[:, g, :],
                            start=(g == 0),
                            stop=(g == CG - 1),
                        )
                    nc.vector.tensor_add(
                        ob[:, oh * QB : (oh + 1) * QB],
                        ps,
                        bias[:, oh * QB : (oh + 1) * QB],
                    )
                    if oh == 1:
                        nc.sync.dma_start(y_h[i * P : (i + 1) * P, :], ob)

                pending = [bias_unit]
                pending += [
                    lambda g4=g4: wo_unit(0, g4) for g4 in (0, 2, 4, 6)
                ]
                pending.append(wo_load1)
                pending += [
                    lambda g4=g4: wo_unit(1, g4) for g4 in (0, 2, 4, 6)
                ]

            nslots = NB * (TG // 2 + 2)
            state = {"slot": 0, "popped": 0}

            def fill():
                state["slot"] += 1
                n = len(pending)
                target = min(n, -(-state["slot"] * n // nslots))
                while state["popped"] < target:
                    pending[state["popped"]]()
                    state["popped"] += 1

            # ---- attention for head pair p ----
            # AV trails the scores/exp stream by 2 key-chunks so exp output
            # is ready when the AV matmul issues; one filler unit per chunk
            # soaks up the exp-rate deficit (exp 2218ns/chunk vs 1720ns of
            # scores+AV matmuls).
            NJ = TG // 2
            for tb in range(NB):
                qa = QT[0:64, tb * QB : (tb + 1) * QB]
                qb = QT[64:128, tb * QB : (tb + 1) * QB]
                pts = []
                ata = at_ps.tile([P, QB], f32, tag="at", name="ata")
                atb = at_ps.tile([P, QB], f32, tag="at", name="atb")

                def av(jj):
                    pta, ptb = pts[jj]
                    for k in range(2):
                        j = 2 * jj + k
                        first, last = (j == 0), (j == TG - 1)
                        nc.tensor.matmul(
                            ata,
                            V[j][:, 192 * p : 192 * p + 128],
                            pta[:, k * QB : (k + 1) * QB],
                            start=first,
                            stop=last,
                        )
                        nc.tensor.matmul(
                            atb,
                            V[j][:, 192 * p + 64 : 192 * p + 192],
                            ptb[:, k * QB : (k + 1) * QB],
                            start=first,
                            stop=last,
                        )

                for jj in range(NJ):
                    sta = st_ps.tile([P, 2 * QB], f32, tag="st", name="sta")
                    stb = st_ps.tile([P, 2 * QB], f32, tag="st", name="stb")
                    for k in range(2):
                        j = 2 * jj + k
                        nc.tensor.matmul(
                            sta[:, k * QB : (k + 1) * QB],
                            KT[0:64, j * P : (j + 1) * P],
                            qa,
                        )
                        nc.tensor.matmul(
                            stb[:, k * QB : (k + 1) * QB],
                            KT[64:128, j * P : (j + 1) * P],
                            qb,
                        )
                    pta = ptp.tile([P, 2 * QB], bf16, tag="pt", name="pta")
                    ptb = ptp.tile([P, 2 * QB], bf16, tag="pt", name="ptb")
                    nc.scalar.activation(pta, sta, AF.Exp)
                    nc.scalar.activation(ptb, stb, AF.Exp)
                    pts.append((pta, ptb))
                    if jj >= 2:
                        av(jj - 2)
                    fill()
                av(NJ - 2)
                fill()
                av(NJ - 1)
                fill()

                combA = recp.tile([64, QB], f32, tag="combA", bufs=1)
                nc.vector.tensor_copy(combA, ata[64:128, :])
                rtA = recp.tile([64, QB], f32, tag="rtA", bufs=1)
                nc.vector.reciprocal_approx_fast(rtA, combA)
                rtB = recp.tile([64, QB], f32, tag="rtB", bufs=1)
                nc.vector.reciprocal_approx_fast(rtB, atb[0:64, :])
                ao = aoT[p][:, tb * QB : (tb + 1) * QB]
                nc.vector.tensor_mul(ao[0:64, :], ata[0:64, :], rtA)
                nc.vector.tensor_mul(ao[64:128, :], atb[64:128, :], rtB)

                if p == NPAIR - 1:
                    # This tb's aoT[7] columns are final now: the out
                    # projection for its 4 token blocks joins the filler
                    # stream (tb0's blocks overlap tb1's attention; tb1's
                    # drain right after).
                    for i in range(tb * 4, tb * 4 + 4):
                        for oh in range(2):
                            pending.append(
                                lambda i=i, oh=oh: op_unit(i, oh)
                            )

            while state["popped"] < len(pending):
                pending[state["popped"]]()
                state["popped"] += 1


def _run(in_maps, trace=False):
    from concourse.bass_utils import run_bass_kernel_spmd

    if "nc" not in _CACHE:
        _CACHE["nc"] = _build()
    nc = _CACHE["nc"]
    return run_bass_kernel_spmd(
        nc, in_maps, core_ids=list(range(8)), trace=trace
    )


def _make_in_maps(x, w_qkv, w_out, b_out):
    x = np.ascontiguousarray(np.asarray(x, dtype=np.float32))
    w_qkv = np.ascontiguousarray(np.asarray(w_qkv, dtype=np.float32))
    w_out = np.ascontiguousarray(np.asarray(w_out, dtype=np.float32))
    b_out = np.ascontiguousarray(np.asarray(b_out, dtype=np.float32))
    in_maps = []
    for c in range(8):
        b, h = divmod(c, 2)
        xb = x[b]
        if h:
            xb = np.ascontiguousarray(
                np.concatenate([xb[TQ:], xb[:TQ]], axis=0)
            )
        in_maps.append(
            {"x": xb, "w_qkv": w_qkv, "w_out": w_out, "b_out": b_out}
        )
    return in_maps


def _gather(results):
    y = np.empty((B, N, C), dtype=np.float32)
    for c in range(8):
        b, h = divmod(c, 2)
        y[b, h * TQ : (h + 1) * TQ, :] = results[c]["y"]
    return y


def kernel(x, w_qkv, w_out, b_out):
    res = _run(_make_in_maps(x, w_qkv, w_out, b_out), trace=False)
    return _gather(res.results)

